# revision 7
# baseline (speedup 1.0000x reference)
"""Trainium2 Bass kernel for ComplexMultiHeadAttention (B=4, S=2048, D=1024, H=16).

Sharding: tensor-parallel over heads across 8 NeuronCores (2 heads/core, all
batches on every core). Each core computes Q/K/V projections for its 2 heads,
full attention for those heads, and a partial output projection against its
128 columns of wo. Partial outputs are summed on the host (the unshard step).

Device-side layout tricks (all host-prepared):
  * X is fed transposed (X^T [D, B*S]) so projections contract over partitions.
  * Q/K projections are built as "stacked" 128-row weight matrices producing
    [q_r ; -q_i] and [k_r ; +k_i] per head directly in PSUM, so
    Re(Q K^T) = one 128-contraction matmul per (k-tile, q-chunk).
  * RoPE head-dim channels are permuted so rotate-half becomes an intra-32-lane
    DVE stream_shuffle; sin sign is folded into the host-built sin table.
  * V is computed transposed then PE-transposed into [s,129] tiles whose last
    column is 1.0, so the attn@V accumulation also produces the softmax
    denominator (no separate reduction). Softmax skips max-subtraction
    (scores are bounded |s| <~ 4 for this problem's data distribution).
"""

import numpy as np

import concourse.bass as bass
import concourse.mybir as mybir
import concourse.tile as tile
from concourse import bacc
from concourse.bass_utils import run_bass_kernel_spmd
from concourse.masks import make_identity

F32 = mybir.dt.float32
P = 128
SC = 512  # s-chunk (matmul moving dim)
HD = 64
D = 1024
NCORES = 8
ROPE_THETA = 10000.0

# rotate-half partner swap within each 32-lane quadrant
SHUF_MASK = list(range(16, 32)) + list(range(0, 16))


def _perm64():
    """Channel permutation: position p (0..63) holds original head-dim dim(p),
    chosen so the rotate-half partner of lane p is lane p^16 (same quadrant)."""
    perm = np.zeros(64, dtype=np.int64)
    for p in range(64):
        q, r = divmod(p, 32)
        perm[p] = q * 16 + r if r < 16 else 32 + q * 16 + (r - 16)
    return perm


PERM64 = _perm64()


def rope_tables(S):
    """cos/sin tables [128, S] matching the permuted stacked layout.
    Row p (p%64 = permuted channel): freq index = q*16 + r%16, sign folded
    into sin (-1 for the first 16 lanes of each quadrant)."""
    inv_freq = 1.0 / (ROPE_THETA ** (np.arange(0, HD, 2, dtype=np.float64) / HD))
    pos = np.arange(S, dtype=np.float64)
    cos_t = np.zeros((P, S), dtype=np.float32)
    sin_t = np.zeros((P, S), dtype=np.float32)
    for p in range(P):
        pl = p % 64
        q, r = divmod(pl, 32)
        fi = q * 16 + (r % 16)
        sign = -1.0 if r < 16 else 1.0
        ang = pos * inv_freq[fi]
        cos_t[p] = np.cos(ang)
        sin_t[p] = sign * np.sin(ang)
    return cos_t, sin_t


def _mm_dt(mm):
    return {"f32": F32, "f32r": F32, "bf16": mybir.dt.bfloat16}[mm]


def _np_dt(mm):
    import ml_dtypes

    return {"f32": np.float32, "f32r": np.float32, "bf16": ml_dtypes.bfloat16}[mm]


def build_program(B, S, mm="f32", reps=1):
    """Build the per-core SPMD program. Returns compiled Bacc."""
    MM = _mm_dt(mm)
    f32r = mybir.dt.float32r

    def mcast(ap):
        # view for matmul operands when using the fp32r fast path
        return ap.bitcast(f32r) if mm == "f32r" else ap

    TP = MM if mm == "bf16" else F32  # transpose psum dtype (must match lhsT)

    NCH = S // SC      # chunks per batch
    KT = S // P        # k-tiles per batch
    BS = B * S

    nc = bacc.Bacc("TRN2", target_bir_lowering=False, debug=False,
                   num_devices=NCORES)

    xr_T = nc.dram_tensor("xr_T", [D, BS], MM, kind="ExternalInput")
    xi_T = nc.dram_tensor("xi_T", [D, BS], MM, kind="ExternalInput")
    wq = nc.dram_tensor("wq", [2, 2, P, 8, P], MM, kind="ExternalInput")
    wk = nc.dram_tensor("wk", [2, 2, P, 8, P], MM, kind="ExternalInput")
    wv = nc.dram_tensor("wv", [2, 2, P, 8, P], MM, kind="ExternalInput")
    wo = nc.dram_tensor("wo", [4, P, D], MM, kind="ExternalInput")
    cos_d = nc.dram_tensor("cos_t", [P, S], F32, kind="ExternalInput")
    sin_d = nc.dram_tensor("sin_t", [P, S], F32, kind="ExternalInput")
    out_rT = nc.dram_tensor("out_rT", [D, BS], F32, kind="ExternalOutput")
    out_iT = nc.dram_tensor("out_iT", [D, BS], F32, kind="ExternalOutput")

    with tile.TileContext(nc) as tc:
        with (
            tc.tile_pool(name="const", bufs=1) as const,
            tc.tile_pool(name="acts", bufs=1) as acts,
            tc.tile_pool(name="work", bufs=2) as work,
            tc.tile_pool(name="psum", bufs=7, space="PSUM") as psum,
        ):
            ident = const.tile([P, P], MM)
            make_identity(nc, ident[:])
            cos_sb = const.tile([P, S], F32)
            sin_sb = const.tile([P, S], F32)
            nc.sync.dma_start(cos_sb[:], cos_d[:])
            nc.sync.dma_start(sin_sb[:], sin_d[:])

            w_sb = {}
            for name, dram in (("q", wq), ("k", wk), ("v", wv)):
                for h in range(2):
                    for t in range(2):
                        wt = const.tile([P, 8, P], MM, tag=f"w{name}{h}{t}")
                        nc.sync.dma_start(wt[:], dram[h, t])
                        w_sb[(name, h, t)] = wt
            wo_sb = []
            for j in range(4):
                wt = const.tile([P, D], MM, tag=f"wo{j}")
                nc.sync.dma_start(wt[:], wo[j])
                wo_sb.append(wt)

            for b in [b_ for _ in range(reps) for b_ in range(B)]:
                QA = acts.tile([P, S], MM, tag="QA")
                QB = acts.tile([P, S], MM, tag="QB")
                KA = acts.tile([P, S], MM, tag="KA")
                KB = acts.tile([P, S], MM, tag="KB")
                VxA = acts.tile([P, KT, 132], MM, tag="VxA")
                VxB = acts.tile([P, KT, 132], MM, tag="VxB")
                MA = acts.tile([P, S], MM, tag="MA")
                MB = acts.tile([P, S], MM, tag="MB")
                nc.vector.memset(VxA[:, :, 128:129], 1.0)
                nc.vector.memset(VxB[:, :, 128:129], 1.0)

                # ---- projections for this batch ----
                for c in range(NCH):
                    col0 = b * S + c * SC
                    cs = c * SC
                    ps_bufs = [psum.tile([P, SC], F32, tag="ps", name=f"prj{k}")
                               for k in range(6)]  # QA QB KA KB VAT VBT
                    mm_w = [("q", 0), ("q", 1), ("k", 0), ("k", 1),
                            ("v", 0), ("v", 1)]
                    for it in range(8):
                        xr_t = work.tile([P, SC], MM, tag="xr", bufs=3)
                        nc.sync.dma_start(
                            xr_t[:], xr_T[it * P:(it + 1) * P, col0:col0 + SC])
                        xi_t = work.tile([P, SC], MM, tag="xi", bufs=3)
                        nc.sync.dma_start(
                            xi_t[:], xi_T[it * P:(it + 1) * P, col0:col0 + SC])
                        for bi, (nm, h) in enumerate(mm_w):
                            nc.tensor.matmul(
                                ps_bufs[bi][:], mcast(w_sb[(nm, h, 0)][:, it, :]),
                                mcast(xr_t[:]), start=(it == 0), stop=False)
                            nc.tensor.matmul(
                                ps_bufs[bi][:], mcast(w_sb[(nm, h, 1)][:, it, :]),
                                mcast(xi_t[:]), start=False, stop=(it == 7))

                    # rope for the four stacked Q/K buffers
                    for ps, buf in zip(ps_bufs[:4], (QA, QB, KA, KB)):
                        sh = work.tile([P, SC], F32, tag="sh", bufs=2)
                        nc.vector.stream_shuffle(sh, ps[:], SHUF_MASK)
                        t2 = work.tile([P, SC], F32, tag="t2", bufs=2)
                        nc.vector.tensor_mul(t2, ps[:], cos_sb[:, cs:cs + SC])
                        nc.vector.tensor_mul(sh, sh, sin_sb[:, cs:cs + SC])
                        nc.vector.tensor_add(buf[:, cs:cs + SC], t2, sh)

                    # V^T -> transpose into [s,129] tiles
                    for ps, Vx in zip(ps_bufs[4:], (VxA, VxB)):
                        vt_sb = work.tile([P, SC], MM, tag="vt", bufs=2)
                        nc.vector.tensor_copy(vt_sb, ps[:])
                        for j in range(4):
                            tp = psum.tile([P, P], TP, tag="ps")
                            nc.tensor.transpose(
                                mcast(tp[:]), mcast(vt_sb[:, j * P:(j + 1) * P]),
                                mcast(ident[:]))
                            nc.vector.tensor_copy(
                                Vx[:, c * 4 + j, 0:128], tp[:])

                # ---- attention for the two heads ----
                for Q, K, Vx, Mh in ((QA, KA, VxA, MA), (QB, KB, VxB, MB)):
                    for qc in range(NCH):
                        o_ps = [psum.tile([P, 132], F32, tag="ps", name=f"ops{j}")
                                for j in range(4)]
                        for kt in range(KT):
                            st_ps = psum.tile([P, SC], F32, tag="ps")
                            nc.tensor.matmul(
                                st_ps[:], mcast(K[:, kt * P:(kt + 1) * P]),
                                mcast(Q[:, qc * SC:(qc + 1) * SC]),
                                start=True, stop=True)
                            st_e = work.tile([P, SC], MM, tag="ste", bufs=3)
                            nc.scalar.activation(
                                st_e, st_ps[:],
                                mybir.ActivationFunctionType.Exp, scale=0.125)
                            for j in range(4):
                                nc.tensor.matmul(
                                    o_ps[j][:, 0:129],
                                    mcast(st_e[:, j * P:(j + 1) * P]),
                                    mcast(Vx[:, kt, 0:129]),
                                    start=(kt == 0), stop=(kt == KT - 1))
                        for j in range(4):
                            rcp = work.tile([P, 1], F32, tag="rcp", bufs=4)
                            nc.vector.reciprocal(rcp, o_ps[j][:, 128:129])
                            o_sb = work.tile([P, P], MM, tag="osb", bufs=3)
                            nc.vector.tensor_scalar_mul(
                                o_sb, o_ps[j][:, 0:128], rcp)
                            qcol = qc * SC + j * P
                            # one transpose -> [128 ch, 128 q] = [Or_h; Oi_h];
                            # stored as-is, r/i recombination folded into wo.
                            tp = psum.tile([P, P], TP, tag="ps")
                            nc.tensor.transpose(
                                mcast(tp[:]), mcast(o_sb[:]), mcast(ident[:]))
                            nc.vector.tensor_copy(
                                Mh[:, qcol:qcol + P], tp[:])

                # ---- partial output projection ----
                for c in range(NCH):
                    col0 = b * S + c * SC
                    cs = c * SC
                    for dt_ in range(8):
                        dsl = slice(dt_ * P, (dt_ + 1) * P)
                        pr = psum.tile([P, SC], F32, tag="ps")
                        nc.tensor.matmul(pr[:], mcast(wo_sb[0][:, dsl]),
                                         mcast(MA[:, cs:cs + SC]),
                                         start=True, stop=False)
                        nc.tensor.matmul(pr[:], mcast(wo_sb[1][:, dsl]),
                                         mcast(MB[:, cs:cs + SC]),
                                         start=False, stop=True)
                        ot = work.tile([P, SC], F32, tag="ot", bufs=3)
                        nc.vector.tensor_copy(ot, pr[:])
                        nc.sync.dma_start(out_rT[dsl, col0:col0 + SC], ot)
                        pi = psum.tile([P, SC], F32, tag="ps")
                        nc.tensor.matmul(pi[:], mcast(wo_sb[2][:, dsl]),
                                         mcast(MA[:, cs:cs + SC]),
                                         start=True, stop=False)
                        nc.tensor.matmul(pi[:], mcast(wo_sb[3][:, dsl]),
                                         mcast(MB[:, cs:cs + SC]),
                                         start=False, stop=True)
                        oti = work.tile([P, SC], F32, tag="oti", bufs=3)
                        nc.vector.tensor_copy(oti, pi[:])
                        nc.sync.dma_start(out_iT[dsl, col0:col0 + SC], oti)

    nc.compile()
    return nc


def prep_core_inputs(core, B, S, mm, x_real, x_imag,
                     wq_r, wq_i, wk_r, wk_i, wv_r, wv_i, wo_r, wo_i,
                     xrT=None, xiT=None, tables=None):
    """Host-side shard prep for one core. xrT/xiT/tables can be shared."""
    npdt = _np_dt(mm)
    if xrT is None:
        xrT = np.ascontiguousarray(
            x_real.reshape(B * S, D).T).astype(npdt)
    if xiT is None:
        xiT = np.ascontiguousarray(
            x_imag.reshape(B * S, D).T).astype(npdt)
    if tables is None:
        tables = rope_tables(S)
    cos_t, sin_t = tables

    def pack_lhsT(mat):  # [1024, 128] -> [128p, 8it, 128m]
        return np.ascontiguousarray(
            mat.reshape(8, P, P).transpose(1, 0, 2)).astype(npdt)

    wq_a = np.zeros((2, 2, P, 8, P), dtype=npdt)
    wk_a = np.zeros((2, 2, P, 8, P), dtype=npdt)
    wv_a = np.zeros((2, 2, P, 8, P), dtype=npdt)
    for h in range(2):
        g = 2 * core + h
        rows = g * HD + PERM64
        Wr_q, Wi_q = wq_r[rows, :], wq_i[rows, :]
        Wr_k, Wi_k = wk_r[rows, :], wk_i[rows, :]
        # stacked Q = [q_r ; -q_i],  K = [k_r ; +k_i]
        Uq = np.vstack([Wr_q, -Wi_q])
        Vq = np.vstack([-Wi_q, -Wr_q])
        Uk = np.vstack([Wr_k, Wi_k])
        Vk = np.vstack([-Wi_k, Wr_k])
        wq_a[h, 0] = pack_lhsT(Uq.T)
        wq_a[h, 1] = pack_lhsT(Vq.T)
        wk_a[h, 0] = pack_lhsT(Uk.T)
        wk_a[h, 1] = pack_lhsT(Vk.T)
        # V^T weights: channels [v_r(64), v_i(64)], natural dim order
        vrows = slice(g * HD, (g + 1) * HD)
        Uv = np.vstack([wv_r[vrows, :], wv_i[vrows, :]])
        Vv = np.vstack([-wv_i[vrows, :], wv_r[vrows, :]])
        wv_a[h, 0] = pack_lhsT(Uv.T)
        wv_a[h, 1] = pack_lhsT(Vv.T)

    wo_a = np.zeros((4, P, D), dtype=npdt)
    for h in range(2):
        g = 2 * core + h
        hs = slice(g * HD, (g + 1) * HD)
        wo_r_h, wo_i_h = wo_r[:, hs], wo_i[:, hs]
        # M_h = [Or_h ; Oi_h]: r-stack gives out_r, i-stack gives out_i
        wo_a[h] = np.vstack([wo_r_h.T, -wo_i_h.T])
        wo_a[2 + h] = np.vstack([wo_i_h.T, wo_r_h.T])

    return {
        "xr_T": xrT, "xi_T": xiT,
        "wq": wq_a, "wk": wk_a, "wv": wv_a, "wo": wo_a,
        "cos_t": cos_t, "sin_t": sin_t,
    }


def prep_all_inputs(B, S, mm, **inputs):
    npdt = _np_dt(mm)
    xrT = np.ascontiguousarray(
        inputs["x_real"].reshape(B * S, D).T).astype(npdt)
    xiT = np.ascontiguousarray(
        inputs["x_imag"].reshape(B * S, D).T).astype(npdt)
    tables = rope_tables(S)
    wargs = {k: inputs[k] for k in
             ("wq_r", "wq_i", "wk_r", "wk_i", "wv_r", "wv_i", "wo_r", "wo_i")}
    return [prep_core_inputs(c, B, S, mm, inputs["x_real"], inputs["x_imag"],
                             xrT=xrT, xiT=xiT, tables=tables, **wargs)
            for c in range(NCORES)]


def combine_outputs(results, B, S):
    """Sum per-core partial transposed outputs, restore [B,S,D] layout."""
    acc_r = np.zeros((D, B * S), dtype=np.float32)
    acc_i = np.zeros((D, B * S), dtype=np.float32)
    for res in results:
        acc_r += res["out_rT"]
        acc_i += res["out_iT"]
    out_r = np.ascontiguousarray(acc_r.T).reshape(B, S, D)
    out_i = np.ascontiguousarray(acc_i.T).reshape(B, S, D)
    return out_r, out_i


_PROGRAM_CACHE = {}


def get_program(B=4, S=2048, mm="f32", reps=1):
    key = (B, S, mm, reps)
    if key not in _PROGRAM_CACHE:
        _PROGRAM_CACHE[key] = build_program(B, S, mm, reps)
    return _PROGRAM_CACHE[key]


MM_MODE = "f32"  # flipped after precision/perf measurement


def kernel(**inputs):
    B, S = 4, 2048
    nc = get_program(B, S, MM_MODE)
    in_maps = prep_all_inputs(B, S, MM_MODE, **inputs)
    res = run_bass_kernel_spmd(nc, in_maps, core_ids=list(range(NCORES)))
    return combine_outputs(res.results, B, S)
